# revision 1
# baseline (speedup 1.0000x reference)
"""LocalVoxelEncoder Trainium2 kernel.

conv3d(1->128, k=3, SAME) + bias + ReLU on x[2,1,64,64,64], then three plane
scatter-means at resolution 128.  The 64-point meshgrid maps injectively into
the 128 plane bins, so each output plane is just the mean over one axis of the
relu'd conv volume, scattered into fixed rows/cols (host-side fancy index).

Sharding: 8 cores = 2 batches x 4 g0-chunks (16 planes each), all 128 channels.
Host pre-builds "x9": the 9 (dx,dy)-shifted, zero-padded copies of the core's
g0-slab, so the whole im2col patch [27 taps, all 16 planes] loads with 4
three-dim HWDGE DMAs (the dz shift and plane seams are just column offsets in
the matmul rhs views).  Per half-plane (g1 split in two for PSUM budget):
  - 4x K=27 fp16 matmuls (lhsT = weights [27,128]) -> psum [128,512] chunks
  - ACT evicts psum with fused bias+ReLU, casting to fp16 c_sb
  - PE identity-matmuls accumulate the yz plane (sum over g0) in PSUM
  - DVE pairwise-tree tensor_adds reduce xz (sum over g1) and xy (sum over g2),
    software-pipelined one plane behind the conv to keep engine streams fed
Host gathers per-core partials, scales by 1/64, scatters into [2,128,128,128].
"""

import os
import sys

import numpy as np

sys.path.insert(0, "/opt/trn_rl_repo")

import concourse.bass as bass
import concourse.bacc as bacc
import concourse.tile as tile
from concourse import mybir
from concourse.bass_utils import run_bass_kernel_spmd

B, C, D = 2, 128, 64
RESO = 128

_g = np.linspace(-0.5, 0.5, D).astype(np.float64)
_xy = np.clip(_g / (1.0 + 0.1 + 10e-4) + 0.5, 0.0, 1.0 - 10e-6)
U = (_xy * RESO).astype(np.int64)  # injective grid-index -> bin map

F16 = mybir.dt.float16
F32 = mybir.dt.float32

_CACHE = {}
LAST_RESULTS = None  # BassKernelResults of the most recent run (for test.py)
LAST_IN_MAPS = None  # per-core input dicts of the most recent run


def _build_nc():
    nc = bacc.Bacc("TRN2", target_bir_lowering=False)
    x9 = nc.dram_tensor("x9", [9, 69760], F16, kind="ExternalInput")
    wkm = nc.dram_tensor("wkm", [27, 128], F16, kind="ExternalInput")
    bias = nc.dram_tensor("bias", [128, 1], F32, kind="ExternalInput")
    ident = nc.dram_tensor("ident", [128, 128], F16, kind="ExternalInput")
    yz_out = nc.dram_tensor("yz_out", [128, 4096], F16, kind="ExternalOutput")
    xz_out = nc.dram_tensor("xz_out", [128, 2048], F16, kind="ExternalOutput")
    xy_out = nc.dram_tensor("xy_out", [128, 1024], F16, kind="ExternalOutput")

    with tile.TileContext(nc) as tc:
        with tc.tile_pool(name="const", bufs=1) as const_pool, \
             tc.tile_pool(name="patch", bufs=1) as patch_pool, \
             tc.tile_pool(name="csb", bufs=4) as csb_pool, \
             tc.tile_pool(name="scr", bufs=4) as scr_pool, \
             tc.tile_pool(name="outs", bufs=1) as out_pool, \
             tc.tile_pool(name="ps", bufs=4, space="PSUM") as ps_pool, \
             tc.tile_pool(name="acc", bufs=1, space="PSUM") as acc_pool:

            wt = const_pool.tile([27, 128], F16)
            nc.sync.dma_start(out=wt[:], in_=wkm[:])
            bi = const_pool.tile([128, 1], F32)
            nc.sync.dma_start(out=bi[:], in_=bias[:])
            idn = const_pool.tile([128, 128], F16)
            nc.sync.dma_start(out=idn[:], in_=ident[:])

            x9_ap = x9[:]
            seg = 3 * 4360 + 4224
            quads = []
            for quad in range(4):
                qt = patch_pool.tile([27, seg], F16, tag=f"q{quad}")
                qt_ap = qt[:]
                qpitch = qt_ap.ap[0][0]
                src = bass.AP(tensor=x9_ap.tensor, offset=quad * 4 * 4360,
                              ap=[[69760, 9], [1, 3], [1, seg]])
                dst = bass.AP(tensor=qt_ap.tensor, offset=qt_ap.offset,
                              ap=[[qpitch, 27], [1, seg]])
                nc.sync.dma_start(out=dst, in_=src)
                quads.append((qt_ap, qpitch))

            xz_sb = out_pool.tile([128, 2048], F16)  # (p:16, h:2, g2:64)
            xy_sb = out_pool.tile([128, 1024], F16)  # (p:16, h:2, g1loc:32)
            yz_sb = out_pool.tile([128, 4096], F16)  # (h:2, g1loc:32, g2:64)


            for h in range(2):
                yz_ps = acc_pool.tile([128, 2048], F32, tag="yzacc")
                prev = None
                for step in range(17):
                    cur = None
                    if step < 16:
                        p = step
                        c_sb = csb_pool.tile([128, 2048], F16)
                        for blk in range(4):
                            ps = ps_pool.tile([128, 512], F32, tag="convps")
                            q_ap, qpitch = quads[p // 4]
                            rhs = bass.AP(
                                tensor=q_ap.tensor,
                                offset=q_ap.offset + (p % 4) * 4360
                                + h * 2112 + blk * 8 * 66,
                                ap=[[qpitch, 27], [66, 8], [1, 64]],
                            )
                            nc.tensor.matmul(
                                ps[:], lhsT=wt[:], rhs=rhs,
                                start=True, stop=True,
                            )
                            nc.scalar.activation(
                                c_sb[:, blk * 512:(blk + 1) * 512], ps[:],
                                mybir.ActivationFunctionType.Relu,
                                bias=bi[:], scale=1.0,
                            )
                        cur = (p, c_sb)

                    if prev is not None:
                        pp, pcsb = prev
                        # yz accumulation over planes (PE identity matmuls)
                        for ss in range(4):
                            nc.tensor.matmul(
                                yz_ps[:, ss * 512:(ss + 1) * 512],
                                lhsT=idn[:],
                                rhs=pcsb[:, ss * 512:(ss + 1) * 512],
                                start=(pp == 0), stop=(pp == 15),
                            )
                        # xz tree: sum over g1 (pairs of g1 half-ranges)
                        s_t = scr_pool.tile([128, 1536], F16, tag="xzscr")
                        nc.vector.tensor_add(s_t[:, 0:1024], pcsb[:, 0:1024], pcsb[:, 1024:2048])
                        nc.vector.tensor_add(s_t[:, 1024:1536], s_t[:, 0:512], s_t[:, 512:1024])
                        nc.vector.tensor_add(s_t[:, 0:256], s_t[:, 1024:1280], s_t[:, 1280:1536])
                        nc.vector.tensor_add(s_t[:, 256:384], s_t[:, 0:128], s_t[:, 128:256])
                        off = pp * 128 + h * 64
                        nc.vector.tensor_add(xz_sb[:, off:off + 64], s_t[:, 256:320], s_t[:, 320:384])
                        # xy tree: sum over g2 within each g1 row
                        t_t = scr_pool.tile([128, 1536], F16, tag="xyscr")
                        c3 = pcsb[:].rearrange("q (a b) -> q a b", a=32)
                        t0 = t_t[:, 0:1024].rearrange("q (a b) -> q a b", a=32)
                        nc.vector.tensor_add(t0, c3[:, :, 0:32], c3[:, :, 32:64])
                        t1 = t_t[:, 1024:1536].rearrange("q (a b) -> q a b", a=32)
                        nc.vector.tensor_add(t1, t0[:, :, 0:16], t0[:, :, 16:32])
                        t2 = t_t[:, 0:256].rearrange("q (a b) -> q a b", a=32)
                        nc.vector.tensor_add(t2, t1[:, :, 0:8], t1[:, :, 8:16])
                        t3 = t_t[:, 256:384].rearrange("q (a b) -> q a b", a=32)
                        nc.vector.tensor_add(t3, t2[:, :, 0:4], t2[:, :, 4:8])
                        t4 = t_t[:, 384:448].rearrange("q (a b) -> q a b", a=32)
                        nc.vector.tensor_add(t4, t3[:, :, 0:2], t3[:, :, 2:4])
                        off = pp * 64 + h * 32
                        nc.vector.tensor_add(
                            xy_sb[:, off:off + 32], t4[:, :, 0], t4[:, :, 1])

                    prev = cur

                nc.scalar.copy(yz_sb[:, h * 2048:(h + 1) * 2048], yz_ps[:])

            nc.sync.dma_start(out=yz_out[:], in_=yz_sb[:])
            nc.sync.dma_start(out=xz_out[:], in_=xz_sb[:])
            nc.sync.dma_start(out=xy_out[:], in_=xy_sb[:])
    nc.compile()
    return nc


def kernel(x, conv_w, conv_b):
    global LAST_RESULTS, LAST_IN_MAPS
    if "nc" not in _CACHE:
        _CACHE["nc"] = _build_nc()
    nc = _CACHE["nc"]

    wkm = np.ascontiguousarray(
        conv_w.reshape(C, 27).T).astype(np.float16)        # [27,128] k=dx*9+dy*3+dz
    bias = conv_b.reshape(C, 1).astype(np.float32)
    ident = np.eye(C, dtype=np.float16)

    in_maps = []
    for core in range(8):
        b, q = core // 4, core % 4
        x_pad = np.pad(x[b, 0], ((1, 1), (1, 3), (1, 1)))  # [66,68,66]
        x9 = np.zeros((9, 16, 4360), np.float16)
        for dx in range(3):
            for dy in range(3):
                blk = x_pad[16 * q + dx:16 * q + dx + 16, dy:dy + 66, :]
                x9[dx * 3 + dy, :, :4356] = blk.reshape(16, 4356)
        in_maps.append({"x9": x9.reshape(9, 69760), "wkm": wkm,
                        "bias": bias, "ident": ident})

    LAST_IN_MAPS = in_maps
    res = run_bass_kernel_spmd(
        nc, in_maps, core_ids=list(range(8)),
        trace=bool(int(os.environ.get("KERNEL_TRACE", "0"))),
    )
    LAST_RESULTS = res

    xz_grid = np.zeros((B, C, 64, 64), np.float32)  # [b, ch, g2, g0]
    xy_grid = np.zeros((B, C, 64, 64), np.float32)  # [b, ch, g1, g0]
    yz_grid = np.zeros((B, C, 64, 64), np.float32)  # [b, ch, g1, g2]
    for core in range(8):
        b, q = core // 4, core % 4
        r = res.results[core]
        xz = r["xz_out"].astype(np.float32).reshape(C, 16, 2, 64)
        xz_grid[b, :, :, 16 * q:16 * q + 16] = (
            xz[:, :, 0, :] + xz[:, :, 1, :]).transpose(0, 2, 1)
        xy = r["xy_out"].astype(np.float32).reshape(C, 16, 64)
        xy_grid[b, :, :, 16 * q:16 * q + 16] = xy.transpose(0, 2, 1)
        yz_grid[b] += r["yz_out"].astype(np.float32).reshape(C, 64, 64)
    xz_grid /= 64.0
    xy_grid /= 64.0
    yz_grid /= 64.0

    fea_xz = np.zeros((B, C, RESO, RESO), np.float32)
    fea_xy = np.zeros((B, C, RESO, RESO), np.float32)
    fea_yz = np.zeros((B, C, RESO, RESO), np.float32)
    rows, cols = U[:, None], U[None, :]
    fea_xz[:, :, rows, cols] = xz_grid
    fea_xy[:, :, rows, cols] = xy_grid
    fea_yz[:, :, rows, cols] = yz_grid.transpose(0, 1, 3, 2)
    return (fea_xz, fea_xy, fea_yz)



# revision 7
# speedup vs baseline: 1.3879x; 1.3879x over previous
"""LocalVoxelEncoder Trainium2 kernel (v2).

conv3d(1->128, k=3, SAME) + bias + ReLU on x[2,1,64,64,64], then three plane
scatter-means at resolution 128.  The 64-point meshgrid maps injectively into
the 128 plane bins, so each output plane is the mean over one axis of the
relu'd conv volume, scattered into fixed rows/cols on the host.

Sharding: 8 cores = 2 batches x 4 g0-quads (16 planes each), all 128 channels.

Per-core schedule (cost-model driven):
  - Host pre-cuts the 27 im2col tap windows per plane into a [128, 16384]
    fp16 SBUF image (4 plane-slots x 32 partitions; taps 0-26 of plane 4u+a
    live on partitions 32a+t, plane-chunk u at columns 4096u).  Input lands
    via 8 [128, 4KB] DMAs on the otherwise-idle SP queue.
  - Conv: K=27 fp16 matmuls, N=512 -> PSUM (2 rotating banks).
  - Eviction: bias+ReLU+cast to fp8e4m3, column-split across ACT
    (activation), Pool and DVE (tensor_scalar add-bias/max-0) so no single
    engine owns the 65536-col stream.
  - Reductions run as fp8 DoubleRow pair-sum matmuls on the PE (identity
    weights duplicated across the two k-tiles sum 2 planes / 2 g1-rows /
    2 g2-cols per pass at half a cycle per output column): yz accumulates
    plane-pairs into a per-h [128,2048] PSUM tile, xz/g1 and xy/g2 pairs
    accumulate into transient 1-bank tiles evicted per half-plane with the
    1/64 mean fold.  A slice of xy is offloaded to Pool/DVE fp16 add-trees
    to balance engine load.
Host sums the per-core yz partials (4 cores per batch) and scatters into the
[2,128,128,128] planes (fixed fancy index).
"""

import os
import sys

import numpy as np

sys.path.insert(0, "/opt/trn_rl_repo")

import concourse.bass as bass
import concourse.bacc as bacc
import concourse.tile as tile
from concourse import mybir
from concourse.bass_utils import run_bass_kernel_spmd

B, C, D = 2, 128, 64
RESO = 128

_g = np.linspace(-0.5, 0.5, D).astype(np.float64)
_xy = np.clip(_g / (1.0 + 0.1 + 10e-4) + 0.5, 0.0, 1.0 - 10e-6)
U = (_xy * RESO).astype(np.int64)  # injective grid-index -> bin map

F16 = mybir.dt.float16
F32 = mybir.dt.float32
F8 = mybir.dt.float8e4
NP_F8 = mybir.dt.np(F8)

_CACHE = {}
LAST_RESULTS = None  # BassKernelResults of the most recent run (for test.py)
LAST_IN_MAPS = None  # per-core input dicts of the most recent run

# --- engine schedules (tuned against the CoreSim cost model) ---------------
# conv-psum eviction engine per 512-col block, pattern over 32 blocks:
#   A=ACT activation, P=Pool tensor_scalar, D=DVE tensor_scalar
_EV_PAT = "AADAADAADAADAADAADAADAADAADAADAD"
# xy reduction placement per half-plane (32 hp): E=PE DoubleRow, P=Pool tree,
# D=DVE tree
_XY_PAT = "PEPD" * 8
# small xz/xy psum eviction engine rotation
_SM_PAT = "DA"


def _build_nc():
    nc = bacc.Bacc("TRN2", target_bir_lowering=False)
    x27 = nc.dram_tensor("x27", [128, 16384], F16, kind="ExternalInput")
    wkm = nc.dram_tensor("wkm", [128, 128], F16, kind="ExternalInput")
    bias = nc.dram_tensor("bias", [128, 1], F32, kind="ExternalInput")
    drw = nc.dram_tensor("drw", [128, 256], F8, kind="ExternalInput")
    yz_out = nc.dram_tensor("yz_out", [128, 4096], F16, kind="ExternalOutput")
    xz_out = nc.dram_tensor("xz_out", [128, 1024], F16, kind="ExternalOutput")
    xy_out = nc.dram_tensor("xy_out", [128, 1024], F16, kind="ExternalOutput")

    inv = 1.0 / 64.0
    ADD, MAX, MULT = (
        mybir.AluOpType.add,
        mybir.AluOpType.max,
        mybir.AluOpType.mult,
    )

    with tile.TileContext(nc) as tc:
        with tc.tile_pool(name="const", bufs=1) as const_pool, \
             tc.tile_pool(name="xin", bufs=1) as xin_pool, \
             tc.tile_pool(name="cp", bufs=4) as c_pool, \
             tc.tile_pool(name="scr", bufs=4) as scr_pool, \
             tc.tile_pool(name="outs", bufs=1) as out_pool, \
             tc.tile_pool(name="cv", bufs=2, space="PSUM") as cv_pool, \
             tc.tile_pool(name="red", bufs=1, space="PSUM") as red_pool, \
             tc.tile_pool(name="yzp", bufs=1, space="PSUM") as yz_pool:

            wt = const_pool.tile([128, 128], F16)
            nc.sync.dma_start(out=wt[:], in_=wkm[:])
            wt_ap = wt[:]
            wpp = wt_ap.ap[0][0]
            bi = const_pool.tile([128, 1], F32)
            nc.sync.dma_start(out=bi[:], in_=bias[:])
            dw = const_pool.tile([128, 256], F8)
            nc.sync.dma_start(out=dw[:], in_=drw[:])

            xt = xin_pool.tile([128, 16384], F16)
            # 8 chunked input DMAs, all on the idle SP queue: (u, h) chunk of
            # 2048 cols covers the h-half of planes 4u..4u+3.
            for h in range(2):
                for u in range(4):
                    c0 = u * 4096 + h * 2048
                    nc.sync.dma_start(
                        out=xt[:, c0:c0 + 2048], in_=x27[:, c0:c0 + 2048])

            xt_ap = xt[:]
            xpp = xt_ap.ap[0][0]
            dw_ap = dw[:]
            dpp = dw_ap.ap[0][0]
            dr_lhs = bass.AP(tensor=dw_ap.tensor, offset=dw_ap.offset,
                             ap=[[dpp, 128], [128, 2], [1, 128]])

            yz_sb = out_pool.tile([128, 4096], F16)  # (h, g1h:32, g2:64)
            xz_sb = out_pool.tile([128, 2048], F16)  # (h, p:16, g2:64)
            xzf = out_pool.tile([128, 1024], F16)    # (p:16, g2:64)
            xy_sb = out_pool.tile([128, 1024], F16)  # (p:16, h, g1h:32)

            def evict(eng, dst, src):
                if eng == "A":
                    nc.scalar.activation(
                        dst, src, mybir.ActivationFunctionType.Relu,
                        bias=bi[:], scale=1.0)
                else:
                    e = nc.gpsimd if eng == "P" else nc.vector
                    e.tensor_scalar(out=dst, in0=src, scalar1=bi[:],
                                    scalar2=0.0, op0=ADD, op1=MAX)

            def evict_scaled(eng, dst, src):
                # psum fp32 -> fp16 with the 1/64 mean fold
                if eng == "A":
                    nc.scalar.activation(
                        dst, src, mybir.ActivationFunctionType.Copy,
                        bias=0.0, scale=inv)
                else:
                    e = nc.gpsimd if eng == "P" else nc.vector
                    e.tensor_scalar(out=dst, in0=src, scalar1=inv,
                                    scalar2=None, op0=MULT)

            def xy_tree(eng, cp_ap, ccols, out_cols):
                # fp8 [128, 32, 64] -> sum over g2 via fp16 pairwise tree
                e = nc.gpsimd if eng == "P" else nc.vector
                s = scr_pool.tile([128, 1536], F16, tag=f"scr{eng}")
                c3 = bass.AP(tensor=cp_ap.tensor, offset=cp_ap.offset + ccols,
                             ap=[[cp_ap.ap[0][0], 128], [64, 32], [1, 64]])
                t0 = s[:, 0:1024].rearrange("q (a b) -> q a b", a=32)
                e.tensor_tensor(out=t0, in0=c3[:, :, 0:32],
                                in1=c3[:, :, 32:64], op=ADD)
                t1 = s[:, 1024:1536].rearrange("q (a b) -> q a b", a=32)
                e.tensor_tensor(out=t1, in0=t0[:, :, 0:16],
                                in1=t0[:, :, 16:32], op=ADD)
                t2 = s[:, 0:256].rearrange("q (a b) -> q a b", a=32)
                e.tensor_tensor(out=t2, in0=t1[:, :, 0:8],
                                in1=t1[:, :, 8:16], op=ADD)
                t3 = s[:, 256:384].rearrange("q (a b) -> q a b", a=32)
                e.tensor_tensor(out=t3, in0=t2[:, :, 0:4],
                                in1=t2[:, :, 4:8], op=ADD)
                t4 = s[:, 384:448].rearrange("q (a b) -> q a b", a=32)
                e.tensor_tensor(out=t4, in0=t3[:, :, 0:2],
                                in1=t3[:, :, 2:4], op=ADD)
                t5 = s[:, 448:480]
                nt4 = s[:, 384:448].rearrange("q (a b) -> q a b", a=32)
                e.tensor_tensor(out=t5, in0=nt4[:, :, 0], in1=nt4[:, :, 1],
                                op=ADD)
                e.tensor_scalar(out=out_cols, in0=t5, scalar1=inv,
                                scalar2=None, op0=MULT)

            ev_i = 0
            sm_i = 0
            pend = []  # deferred per-hp reduction emitters (1-hp lookahead)
            for h in range(2):
                yz_ps = yz_pool.tile([128, 2048], F32, tag="yz")

                for p in range(16):
                    u, a = p // 4, p % 4
                    if p % 2 == 0:
                        cpt = c_pool.tile([128, 4096], F8, tag="cpair")
                        cpt_ap = cpt[:]
                        cpp = cpt_ap.ap[0][0]
                    ccols = (p % 2) * 2048

                    for blk in range(4):
                        ps = cv_pool.tile([128, 512], F32, tag="cv")
                        rhs = bass.AP(
                            tensor=xt_ap.tensor,
                            offset=xt_ap.offset + 32 * a * xpp + u * 4096
                            + (h * 32 + blk * 8) * 64,
                            ap=[[xpp, 27], [64, 8], [1, 64]],
                        )
                        lhs = bass.AP(
                            tensor=wt_ap.tensor,
                            offset=wt_ap.offset + 32 * a * wpp,
                            ap=[[wpp, 27], [1, 128]],
                        )
                        nc.tensor.matmul(ps[:], lhsT=lhs, rhs=rhs,
                                         start=True, stop=True,
                                         tile_position=(32 * a, 0))
                        dst = cpt[:, ccols + blk * 512:ccols + (blk + 1) * 512]
                        evict(_EV_PAT[ev_i % len(_EV_PAT)], dst, ps[:])
                        ev_i += 1

                    def make_red(h=h, p=p, cpt_ap=cpt_ap, cpp=cpp,
                                 ccols=ccols, yz_ps=yz_ps):
                        def emit():
                            nonlocal sm_i
                            hp = h * 16 + p
                            # xz: sum over the 16 g1-pairs of this half-plane
                            xz_t = red_pool.tile([128, 512], F32, tag="redxz")
                            for j in range(16):
                                rhs = bass.AP(
                                    tensor=cpt_ap.tensor,
                                    offset=cpt_ap.offset + ccols + j * 128,
                                    ap=[[cpp, 128], [64, 2], [1, 64]])
                                nc.tensor.matmul(
                                    xz_t[:, 0:64], lhsT=dr_lhs, rhs=rhs,
                                    start=(j == 0), stop=(j == 15),
                                    perf_mode=mybir.MatmulPerfMode.DoubleRow)
                            evict_scaled(
                                _SM_PAT[sm_i % 2],
                                xz_sb[:, h * 1024 + p * 64:h * 1024 + p * 64 + 64],
                                xz_t[:, 0:64])
                            sm_i += 1
                            # xy: sum over g2 for the 32 g1-rows
                            xy_dst = xy_sb[:, p * 64 + h * 32:p * 64 + h * 32 + 32]
                            mode = _XY_PAT[hp]
                            if mode == "E":
                                xy_t = red_pool.tile([128, 512], F32,
                                                     tag="redxy")
                                for g in range(32):
                                    rhs = bass.AP(
                                        tensor=cpt_ap.tensor,
                                        offset=cpt_ap.offset + ccols + 2 * g,
                                        ap=[[cpp, 128], [1, 2], [64, 32]])
                                    nc.tensor.matmul(
                                        xy_t[:, 0:32], lhsT=dr_lhs, rhs=rhs,
                                        start=(g == 0), stop=(g == 31),
                                        perf_mode=mybir.MatmulPerfMode.DoubleRow)
                                evict_scaled(_SM_PAT[sm_i % 2], xy_dst,
                                             xy_t[:, 0:32])
                                sm_i += 1
                            else:
                                xy_tree(mode, cpt_ap, ccols, xy_dst)
                            # yz: plane-pair accumulation on odd planes
                            if p % 2 == 1:
                                for ss in range(4):
                                    rhs = bass.AP(
                                        tensor=cpt_ap.tensor,
                                        offset=cpt_ap.offset + ss * 512,
                                        ap=[[cpp, 128], [2048, 2], [1, 512]])
                                    nc.tensor.matmul(
                                        yz_ps[:, ss * 512:(ss + 1) * 512],
                                        lhsT=dr_lhs, rhs=rhs,
                                        start=(p == 1), stop=(p == 15),
                                        perf_mode=mybir.MatmulPerfMode.DoubleRow)
                        return emit

                    pend.append(make_red())
                    if len(pend) > 1:
                        pend.pop(0)()
                while pend:
                    pend.pop(0)()

                # yz eviction for this h, split across engines
                ybase = h * 2048
                nc.scalar.activation(
                    yz_sb[:, ybase:ybase + 1024], yz_ps[:, 0:1024],
                    mybir.ActivationFunctionType.Copy, bias=0.0, scale=inv)
                nc.vector.tensor_scalar(
                    out=yz_sb[:, ybase + 1024:ybase + 2048],
                    in0=yz_ps[:, 1024:2048], scalar1=inv, scalar2=None,
                    op0=MULT)
                nc.sync.dma_start(out=yz_out[:, ybase:ybase + 2048],
                                  in_=yz_sb[:, ybase:ybase + 2048])

            # xz: combine the two g1-halves (already 1/64-scaled)
            nc.vector.tensor_tensor(out=xzf[:], in0=xz_sb[:, 0:1024],
                                    in1=xz_sb[:, 1024:2048], op=ADD)
            nc.sync.dma_start(out=xz_out[:], in_=xzf[:])
            nc.sync.dma_start(out=xy_out[:], in_=xy_sb[:])
    nc.compile()
    return nc


def _host_inputs(x, conv_w, conv_b):
    w27 = np.ascontiguousarray(
        conv_w.reshape(C, 27).T).astype(np.float16)      # [27,128] t=dx*9+dy*3+dz
    wkm = np.zeros((128, 128), np.float16)
    for a in range(4):
        wkm[32 * a:32 * a + 27] = w27
    bias = conv_b.reshape(C, 1).astype(np.float32)
    drw = np.zeros((128, 256), NP_F8)
    idx = np.arange(128)
    drw[idx, idx] = 1.0
    drw[idx, 128 + idx] = 1.0

    in_maps = []
    for core in range(8):
        b, q = core // 4, core % 4
        xe = np.zeros((18, 66, 66), np.float32)
        lo = 16 * q - 1
        s0, e0 = max(lo, 0), min(lo + 18, 64)
        xe[s0 - lo:s0 - lo + (e0 - s0), 1:65, 1:65] = x[b, 0, s0:e0]
        x27 = np.zeros((128, 16384), np.float16)
        for a in range(4):
            for t in range(27):
                dx, r = divmod(t, 9)
                dy, dz = divmod(r, 3)
                row = 32 * a + t
                for u in range(4):
                    p = 4 * u + a
                    x27[row, 4096 * u:4096 * (u + 1)] = (
                        xe[p + dx, dy:dy + 64, dz:dz + 64].reshape(-1))
        in_maps.append({"x27": x27, "wkm": wkm, "bias": bias, "drw": drw})
    return in_maps


def kernel(x, conv_w, conv_b):
    global LAST_RESULTS, LAST_IN_MAPS
    if "nc" not in _CACHE:
        _CACHE["nc"] = _build_nc()
    nc = _CACHE["nc"]

    in_maps = _host_inputs(x, conv_w, conv_b)
    LAST_IN_MAPS = in_maps
    res = run_bass_kernel_spmd(
        nc, in_maps, core_ids=list(range(8)),
        trace=bool(int(os.environ.get("KERNEL_TRACE", "0"))),
    )
    LAST_RESULTS = res

    xz_grid = np.zeros((B, C, 64, 64), np.float32)  # [b, ch, g2, g0]
    xy_grid = np.zeros((B, C, 64, 64), np.float32)  # [b, ch, g1, g0]
    yz_grid = np.zeros((B, C, 64, 64), np.float32)  # [b, ch, g1, g2]
    for core in range(8):
        b, q = core // 4, core % 4
        r = res.results[core]
        xz = r["xz_out"].astype(np.float32).reshape(C, 16, 64)  # [ch,p,g2]
        xz_grid[b, :, :, 16 * q:16 * q + 16] = xz.transpose(0, 2, 1)
        xy = r["xy_out"].astype(np.float32).reshape(C, 16, 64)  # [ch,p,g1]
        xy_grid[b, :, :, 16 * q:16 * q + 16] = xy.transpose(0, 2, 1)
        yz = r["yz_out"].astype(np.float32).reshape(C, 64, 64)  # [ch,g1,g2]
        yz_grid[b] += yz

    fea_xz = np.zeros((B, C, RESO, RESO), np.float32)
    fea_xy = np.zeros((B, C, RESO, RESO), np.float32)
    fea_yz = np.zeros((B, C, RESO, RESO), np.float32)
    rows, cols = U[:, None], U[None, :]
    fea_xz[:, :, rows, cols] = xz_grid
    fea_xy[:, :, rows, cols] = xy_grid
    fea_yz[:, :, rows, cols] = yz_grid.transpose(0, 1, 3, 2)
    return (fea_xz, fea_xy, fea_yz)


# revision 8
# speedup vs baseline: 1.6019x; 1.1542x over previous
"""LocalVoxelEncoder Trainium2 kernel (v2).

conv3d(1->128, k=3, SAME) + bias + ReLU on x[2,1,64,64,64], then three plane
scatter-means at resolution 128.  The 64-point meshgrid maps injectively into
the 128 plane bins, so each output plane is the mean over one axis of the
relu'd conv volume, scattered into fixed rows/cols on the host.

Sharding: 8 cores = 2 batches x 4 g0-quads (16 planes each), all 128 channels.

Per-core schedule (cost-model driven):
  - Host pre-cuts the 27 im2col tap windows per plane into a [128, 16384]
    fp16 SBUF image (4 plane-slots x 32 partitions; taps 0-26 of plane 4u+a
    live on partitions 32a+t, plane-chunk u at columns 4096u).  Input lands
    via 8 [128, 4KB] DMAs on the otherwise-idle SP queue.
  - Conv: K=27 fp16 matmuls, N=512 -> PSUM (2 rotating banks).
  - Eviction: bias+ReLU+cast to fp8e4m3, column-split across ACT
    (activation), Pool and DVE (tensor_scalar add-bias/max-0) so no single
    engine owns the 65536-col stream.
  - Reductions run as fp8 DoubleRow pair-sum matmuls on the PE (identity
    weights duplicated across the two k-tiles sum 2 planes / 2 g1-rows /
    2 g2-cols per pass at half a cycle per output column): yz accumulates
    plane-pairs into a per-h [128,2048] PSUM tile, xz/g1 and xy/g2 pairs
    accumulate into transient 1-bank tiles evicted per half-plane with the
    1/64 mean fold.  A slice of xy is offloaded to Pool/DVE fp16 add-trees
    to balance engine load.
Host sums the per-core yz partials (4 cores per batch) and scatters into the
[2,128,128,128] planes (fixed fancy index).
"""

import os
import sys

import numpy as np

sys.path.insert(0, "/opt/trn_rl_repo")

import concourse.bass as bass
import concourse.bacc as bacc
import concourse.tile as tile
from concourse import mybir
from concourse.bass_utils import run_bass_kernel_spmd

B, C, D = 2, 128, 64
RESO = 128

_g = np.linspace(-0.5, 0.5, D).astype(np.float64)
_xy = np.clip(_g / (1.0 + 0.1 + 10e-4) + 0.5, 0.0, 1.0 - 10e-6)
U = (_xy * RESO).astype(np.int64)  # injective grid-index -> bin map

F16 = mybir.dt.float16
F32 = mybir.dt.float32
F8 = mybir.dt.float8e4
NP_F8 = mybir.dt.np(F8)

_CACHE = {}
LAST_RESULTS = None  # BassKernelResults of the most recent run (for test.py)
LAST_IN_MAPS = None  # per-core input dicts of the most recent run

# --- engine schedules (tuned against the CoreSim cost model) ---------------
# conv-psum eviction engine per 512-col block, pattern over 32 blocks:
#   A=ACT activation, P=Pool tensor_scalar, D=DVE tensor_scalar
_EV_PAT = "AADAADAADAADAADAADAADAADAADAADAD"
# xy reduction placement per half-plane (32 hp): E=PE DoubleRow, P=Pool tree,
# D=DVE tree
_XY_PAT = "PPEPPPDP" "PPEPPPDP" "PPEPPPDP" "PEEPPPDP"
# small xz/xy psum eviction engine rotation
_SM_PAT = "DA"


def _build_nc():
    nc = bacc.Bacc("TRN2", target_bir_lowering=False)
    x27 = nc.dram_tensor("x27", [128, 16384], F16, kind="ExternalInput")
    wkm = nc.dram_tensor("wkm", [128, 128], F16, kind="ExternalInput")
    bias = nc.dram_tensor("bias", [128, 1], F32, kind="ExternalInput")
    drw = nc.dram_tensor("drw", [128, 256], F8, kind="ExternalInput")
    yz_out = nc.dram_tensor("yz_out", [128, 4096], F16, kind="ExternalOutput")
    xz_out = nc.dram_tensor("xz_out", [128, 1024], F16, kind="ExternalOutput")
    xy_out = nc.dram_tensor("xy_out", [128, 1024], F16, kind="ExternalOutput")

    inv = 1.0 / 64.0
    ADD, MAX, MULT = (
        mybir.AluOpType.add,
        mybir.AluOpType.max,
        mybir.AluOpType.mult,
    )

    with tile.TileContext(nc) as tc:
        with tc.tile_pool(name="const", bufs=1) as const_pool, \
             tc.tile_pool(name="xin", bufs=1) as xin_pool, \
             tc.tile_pool(name="cp", bufs=16) as c_pool, \
             tc.tile_pool(name="scr", bufs=4) as scr_pool, \
             tc.tile_pool(name="outs", bufs=1) as out_pool, \
             tc.tile_pool(name="cv", bufs=2, space="PSUM") as cv_pool, \
             tc.tile_pool(name="red", bufs=1, space="PSUM") as red_pool, \
             tc.tile_pool(name="yzp", bufs=1, space="PSUM") as yz_pool:

            wt = const_pool.tile([128, 128], F16)
            nc.sync.dma_start(out=wt[:], in_=wkm[:])
            wt_ap = wt[:]
            wpp = wt_ap.ap[0][0]
            bi = const_pool.tile([128, 1], F32)
            nc.sync.dma_start(out=bi[:], in_=bias[:])
            dw = const_pool.tile([128, 256], F8)
            nc.sync.dma_start(out=dw[:], in_=drw[:])

            xt = xin_pool.tile([128, 16384], F16)
            # 8 chunked input DMAs, all on the idle SP queue: (u, h) chunk of
            # 2048 cols covers the h-half of planes 4u..4u+3.
            for h in range(2):
                for u in range(4):
                    c0 = u * 4096 + h * 2048
                    nc.sync.dma_start(
                        out=xt[:, c0:c0 + 2048], in_=x27[:, c0:c0 + 2048])

            xt_ap = xt[:]
            xpp = xt_ap.ap[0][0]
            dw_ap = dw[:]
            dpp = dw_ap.ap[0][0]
            dr_lhs = bass.AP(tensor=dw_ap.tensor, offset=dw_ap.offset,
                             ap=[[dpp, 128], [128, 2], [1, 128]])

            yz_sb = out_pool.tile([128, 4096], F16)  # (h, g1h:32, g2:64)
            xz_sb = out_pool.tile([128, 2048], F16)  # (h, p:16, g2:64)
            xzf = out_pool.tile([128, 1024], F16)    # (p:16, g2:64)
            xy_sb = out_pool.tile([128, 1024], F16)  # (p:16, h, g1h:32)

            def evict(eng, dst, src):
                if eng == "A":
                    nc.scalar.activation(
                        dst, src, mybir.ActivationFunctionType.Relu,
                        bias=bi[:], scale=1.0)
                else:
                    e = nc.gpsimd if eng == "P" else nc.vector
                    e.tensor_scalar(out=dst, in0=src, scalar1=bi[:],
                                    scalar2=0.0, op0=ADD, op1=MAX)

            def evict_scaled(eng, dst, src):
                # psum fp32 -> fp16 with the 1/64 mean fold
                if eng == "A":
                    nc.scalar.activation(
                        dst, src, mybir.ActivationFunctionType.Copy,
                        bias=0.0, scale=inv)
                else:
                    e = nc.gpsimd if eng == "P" else nc.vector
                    e.tensor_scalar(out=dst, in0=src, scalar1=inv,
                                    scalar2=None, op0=MULT)

            def xy_tree(eng, cp_ap, ccols, out_cols):
                # fp8 [128, 32, 64] -> sum over g2 via fp16 pairwise tree
                e = nc.gpsimd if eng == "P" else nc.vector
                s = scr_pool.tile([128, 1536], F16, tag=f"scr{eng}")
                c3 = bass.AP(tensor=cp_ap.tensor, offset=cp_ap.offset + ccols,
                             ap=[[cp_ap.ap[0][0], 128], [64, 32], [1, 64]])
                t0 = s[:, 0:1024].rearrange("q (a b) -> q a b", a=32)
                e.tensor_tensor(out=t0, in0=c3[:, :, 0:32],
                                in1=c3[:, :, 32:64], op=ADD)
                t1 = s[:, 1024:1536].rearrange("q (a b) -> q a b", a=32)
                e.tensor_tensor(out=t1, in0=t0[:, :, 0:16],
                                in1=t0[:, :, 16:32], op=ADD)
                t2 = s[:, 0:256].rearrange("q (a b) -> q a b", a=32)
                e.tensor_tensor(out=t2, in0=t1[:, :, 0:8],
                                in1=t1[:, :, 8:16], op=ADD)
                t3 = s[:, 256:384].rearrange("q (a b) -> q a b", a=32)
                e.tensor_tensor(out=t3, in0=t2[:, :, 0:4],
                                in1=t2[:, :, 4:8], op=ADD)
                t4 = s[:, 384:448].rearrange("q (a b) -> q a b", a=32)
                e.tensor_tensor(out=t4, in0=t3[:, :, 0:2],
                                in1=t3[:, :, 2:4], op=ADD)
                t5 = s[:, 448:480]
                nt4 = s[:, 384:448].rearrange("q (a b) -> q a b", a=32)
                e.tensor_tensor(out=t5, in0=nt4[:, :, 0], in1=nt4[:, :, 1],
                                op=ADD)
                e.tensor_scalar(out=out_cols, in0=t5, scalar1=inv,
                                scalar2=None, op0=MULT)

            # Bresenham A/D rotation for the 64 conv-psum eviction chunks
            EV_ACT = 34
            ev_engines = []
            acc = 0
            for i in range(64):
                acc += EV_ACT
                if acc >= 64:
                    acc -= 64
                    ev_engines.append("A")
                else:
                    ev_engines.append("D")

            ev_i = 0
            pend = []  # deferred per-hp reduction emitters (2-hp lookahead)
            pair_tiles = {}
            for h in range(2):
                for p in range(16):
                    u, a = p // 4, p % 4
                    g4 = (h * 16 + p) // 4   # 4-hp reduction bank group
                    r4 = (h * 16 + p) % 4
                    if r4 == 0:
                        xz_bank = red_pool.tile([128, 512], F32, tag="redxz")
                        xy_bank = red_pool.tile([128, 512], F32, tag="redxy")
                    if p % 2 == 0:
                        cpt = c_pool.tile([128, 4096], F8, tag="cpair")
                        pair_tiles[(h, p // 2)] = cpt
                    cpt_ap = pair_tiles[(h, p // 2)][:]
                    cpp = cpt_ap.ap[0][0]
                    ccols = (p % 2) * 2048

                    for blk2 in range(2):
                        ps = cv_pool.tile([128, 1024], F32, tag="cv")
                        for half in range(2):
                            blk = blk2 * 2 + half
                            rhs = bass.AP(
                                tensor=xt_ap.tensor,
                                offset=xt_ap.offset + 32 * a * xpp + u * 4096
                                + (h * 32 + blk * 8) * 64,
                                ap=[[xpp, 27], [64, 8], [1, 64]],
                            )
                            lhs = bass.AP(
                                tensor=wt_ap.tensor,
                                offset=wt_ap.offset + 32 * a * wpp,
                                ap=[[wpp, 27], [1, 128]],
                            )
                            nc.tensor.matmul(
                                ps[:, half * 512:(half + 1) * 512],
                                lhsT=lhs, rhs=rhs, start=True, stop=True,
                                tile_position=(32 * a, 0))
                        dst = cpt_ap.tensor  # noqa: placeholder
                        dstap = pair_tiles[(h, p // 2)][
                            :, ccols + blk2 * 1024:ccols + (blk2 + 1) * 1024]
                        evict(ev_engines[ev_i % 64], dstap, ps[:])
                        ev_i += 1

                    def make_red(h=h, p=p, cpt_ap=cpt_ap, cpp=cpp,
                                 ccols=ccols, xz_bank=xz_bank,
                                 xy_bank=xy_bank, r4=r4):
                        def emit():
                            hp = h * 16 + p
                            # xz: 16 g1-pair matmuls into the shared 4-hp bank
                            for j in range(16):
                                rhs = bass.AP(
                                    tensor=cpt_ap.tensor,
                                    offset=cpt_ap.offset + ccols + j * 128,
                                    ap=[[cpp, 128], [64, 2], [1, 64]])
                                nc.tensor.matmul(
                                    xz_bank[:, r4 * 64:r4 * 64 + 64],
                                    lhsT=dr_lhs, rhs=rhs,
                                    start=(r4 == 0 and j == 0),
                                    stop=(r4 == 3 and j == 15),
                                    perf_mode=mybir.MatmulPerfMode.DoubleRow)
                            # xy
                            xy_dst = xy_sb[:, p * 64 + h * 32:
                                           p * 64 + h * 32 + 32]
                            mode = _XY_PAT[hp]
                            if mode == "E":
                                for g in range(32):
                                    rhs = bass.AP(
                                        tensor=cpt_ap.tensor,
                                        offset=cpt_ap.offset + ccols + 2 * g,
                                        ap=[[cpp, 128], [1, 2], [64, 32]])
                                    nc.tensor.matmul(
                                        xy_bank[:, r4 * 32:r4 * 32 + 32],
                                        lhsT=dr_lhs, rhs=rhs,
                                        start=(g == 0), stop=(g == 31),
                                        perf_mode=mybir.MatmulPerfMode.DoubleRow)
                            else:
                                xy_tree(mode, cpt_ap, ccols, xy_dst)
                            # yz part A (cols 0-1023) incremental on odd planes
                            if p % 2 == 1:
                                for ss in range(2):
                                    rhs = bass.AP(
                                        tensor=cpt_ap.tensor,
                                        offset=cpt_ap.offset + ss * 512,
                                        ap=[[cpp, 128], [2048, 2], [1, 512]])
                                    nc.tensor.matmul(
                                        yz_psA[:, ss * 512:(ss + 1) * 512],
                                        lhsT=dr_lhs, rhs=rhs,
                                        start=(p == 1), stop=(p == 15),
                                        perf_mode=mybir.MatmulPerfMode.DoubleRow)
                        return emit

                    if p == 0:
                        yz_psA = yz_pool.tile([128, 1024], F32, tag="yzA")
                    pend.append(make_red())
                    while len(pend) > 2:
                        pend.pop(0)()

                    if r4 == 3:
                        # defer bank evictions behind the lookahead
                        def make_bank_ev(h=h, g4=g4, xz_bank=xz_bank,
                                         xy_bank=xy_bank):
                            def emit():
                                base = g4 * 4 * 64 - (h * 16 * 64) + h * 1024
                                evict_scaled(
                                    "D",
                                    xz_sb[:, h * 1024 + (g4 % 4) * 256:
                                          h * 1024 + (g4 % 4) * 256 + 256],
                                    xz_bank[:, 0:256])
                                # xy PE-mode columns: copy out any E hps
                                for k in range(4):
                                    hp = g4 * 4 + k
                                    if _XY_PAT[hp] == "E":
                                        pp_ = hp % 16
                                        hh = hp // 16
                                        evict_scaled(
                                            "D",
                                            xy_sb[:, pp_ * 64 + hh * 32:
                                                  pp_ * 64 + hh * 32 + 32],
                                            xy_bank[:, k * 32:k * 32 + 32])
                            return emit
                        pend.append(make_bank_ev())

                while pend:
                    pend.pop(0)()

                # yz part B (cols 1024-2047): re-read the 8 resident pairs
                yz_psB = yz_pool.tile([128, 1024], F32, tag="yzA")
                for k in range(8):
                    cpt_ap = pair_tiles[(h, k)][:]
                    cpp = cpt_ap.ap[0][0]
                    for ss in range(2):
                        rhs = bass.AP(
                            tensor=cpt_ap.tensor,
                            offset=cpt_ap.offset + 1024 + ss * 512,
                            ap=[[cpp, 128], [2048, 2], [1, 512]])
                        nc.tensor.matmul(
                            yz_psB[:, ss * 512:(ss + 1) * 512],
                            lhsT=dr_lhs, rhs=rhs,
                            start=(k == 0), stop=(k == 7),
                            perf_mode=mybir.MatmulPerfMode.DoubleRow)

                # yz evictions for this h
                ybase = h * 2048
                nc.scalar.activation(
                    yz_sb[:, ybase:ybase + 1024], yz_psA[:],
                    mybir.ActivationFunctionType.Copy, bias=0.0, scale=inv)
                nc.scalar.activation(
                    yz_sb[:, ybase + 1024:ybase + 2048], yz_psB[:],
                    mybir.ActivationFunctionType.Copy, bias=0.0, scale=inv)
                nc.sync.dma_start(out=yz_out[:, ybase:ybase + 2048],
                                  in_=yz_sb[:, ybase:ybase + 2048])
                pair_tiles.clear()

            # xz: combine the two g1-halves (already 1/64-scaled)
            nc.vector.tensor_tensor(out=xzf[:], in0=xz_sb[:, 0:1024],
                                    in1=xz_sb[:, 1024:2048], op=ADD)
            nc.sync.dma_start(out=xz_out[:], in_=xzf[:])
            nc.sync.dma_start(out=xy_out[:], in_=xy_sb[:])
    nc.compile()
    return nc


def _host_inputs(x, conv_w, conv_b):
    w27 = np.ascontiguousarray(
        conv_w.reshape(C, 27).T).astype(np.float16)      # [27,128] t=dx*9+dy*3+dz
    wkm = np.zeros((128, 128), np.float16)
    for a in range(4):
        wkm[32 * a:32 * a + 27] = w27
    bias = conv_b.reshape(C, 1).astype(np.float32)
    drw = np.zeros((128, 256), NP_F8)
    idx = np.arange(128)
    drw[idx, idx] = 1.0
    drw[idx, 128 + idx] = 1.0

    in_maps = []
    for core in range(8):
        b, q = core // 4, core % 4
        xe = np.zeros((18, 66, 66), np.float32)
        lo = 16 * q - 1
        s0, e0 = max(lo, 0), min(lo + 18, 64)
        xe[s0 - lo:s0 - lo + (e0 - s0), 1:65, 1:65] = x[b, 0, s0:e0]
        x27 = np.zeros((128, 16384), np.float16)
        for a in range(4):
            for t in range(27):
                dx, r = divmod(t, 9)
                dy, dz = divmod(r, 3)
                row = 32 * a + t
                for u in range(4):
                    p = 4 * u + a
                    x27[row, 4096 * u:4096 * (u + 1)] = (
                        xe[p + dx, dy:dy + 64, dz:dz + 64].reshape(-1))
        in_maps.append({"x27": x27, "wkm": wkm, "bias": bias, "drw": drw})
    return in_maps


def kernel(x, conv_w, conv_b):
    global LAST_RESULTS, LAST_IN_MAPS
    if "nc" not in _CACHE:
        _CACHE["nc"] = _build_nc()
    nc = _CACHE["nc"]

    in_maps = _host_inputs(x, conv_w, conv_b)
    LAST_IN_MAPS = in_maps
    res = run_bass_kernel_spmd(
        nc, in_maps, core_ids=list(range(8)),
        trace=bool(int(os.environ.get("KERNEL_TRACE", "0"))),
    )
    LAST_RESULTS = res

    xz_grid = np.zeros((B, C, 64, 64), np.float32)  # [b, ch, g2, g0]
    xy_grid = np.zeros((B, C, 64, 64), np.float32)  # [b, ch, g1, g0]
    yz_grid = np.zeros((B, C, 64, 64), np.float32)  # [b, ch, g1, g2]
    for core in range(8):
        b, q = core // 4, core % 4
        r = res.results[core]
        xz = r["xz_out"].astype(np.float32).reshape(C, 16, 64)  # [ch,p,g2]
        xz_grid[b, :, :, 16 * q:16 * q + 16] = xz.transpose(0, 2, 1)
        xy = r["xy_out"].astype(np.float32).reshape(C, 16, 64)  # [ch,p,g1]
        xy_grid[b, :, :, 16 * q:16 * q + 16] = xy.transpose(0, 2, 1)
        yz = r["yz_out"].astype(np.float32).reshape(C, 64, 64)  # [ch,g1,g2]
        yz_grid[b] += yz

    fea_xz = np.zeros((B, C, RESO, RESO), np.float32)
    fea_xy = np.zeros((B, C, RESO, RESO), np.float32)
    fea_yz = np.zeros((B, C, RESO, RESO), np.float32)
    rows, cols = U[:, None], U[None, :]
    fea_xz[:, :, rows, cols] = xz_grid
    fea_xy[:, :, rows, cols] = xy_grid
    fea_yz[:, :, rows, cols] = yz_grid.transpose(0, 1, 3, 2)
    return (fea_xz, fea_xy, fea_yz)


# revision 10
# speedup vs baseline: 1.6920x; 1.0563x over previous
"""LocalVoxelEncoder Trainium2 kernel (v2).

conv3d(1->128, k=3, SAME) + bias + ReLU on x[2,1,64,64,64], then three plane
scatter-means at resolution 128.  The 64-point meshgrid maps injectively into
the 128 plane bins, so each output plane is the mean over one axis of the
relu'd conv volume, scattered into fixed rows/cols on the host.

Sharding: 8 cores = 2 batches x 4 g0-quads (16 planes each), all 128 channels.

Per-core schedule (cost-model driven):
  - Host pre-cuts the 27 im2col tap windows per plane into a [128, 16384]
    fp16 SBUF image (4 plane-slots x 32 partitions; taps 0-26 of plane 4u+a
    live on partitions 32a+t, plane-chunk u at columns 4096u).  Input lands
    via 8 [128, 4KB] DMAs on the otherwise-idle SP queue.
  - Conv: K=27 fp16 matmuls, N=512 -> PSUM (2 rotating banks).
  - Eviction: bias+ReLU+cast to fp8e4m3, column-split across ACT
    (activation), Pool and DVE (tensor_scalar add-bias/max-0) so no single
    engine owns the 65536-col stream.
  - Reductions run as fp8 DoubleRow pair-sum matmuls on the PE (identity
    weights duplicated across the two k-tiles sum 2 planes / 2 g1-rows /
    2 g2-cols per pass at half a cycle per output column): yz accumulates
    plane-pairs into a per-h [128,2048] PSUM tile, xz/g1 and xy/g2 pairs
    accumulate into transient 1-bank tiles evicted per half-plane with the
    1/64 mean fold.  A slice of xy is offloaded to Pool/DVE fp16 add-trees
    to balance engine load.
Host sums the per-core yz partials (4 cores per batch) and scatters into the
[2,128,128,128] planes (fixed fancy index).
"""

import os
import sys

import numpy as np

sys.path.insert(0, "/opt/trn_rl_repo")

import concourse.bass as bass
import concourse.bacc as bacc
import concourse.tile as tile
from concourse import mybir
from concourse.bass_utils import run_bass_kernel_spmd

B, C, D = 2, 128, 64
RESO = 128

_g = np.linspace(-0.5, 0.5, D).astype(np.float64)
_xy = np.clip(_g / (1.0 + 0.1 + 10e-4) + 0.5, 0.0, 1.0 - 10e-6)
U = (_xy * RESO).astype(np.int64)  # injective grid-index -> bin map

F16 = mybir.dt.float16
F32 = mybir.dt.float32
F8 = mybir.dt.float8e4
NP_F8 = mybir.dt.np(F8)

_CACHE = {}
LAST_RESULTS = None  # BassKernelResults of the most recent run (for test.py)
LAST_IN_MAPS = None  # per-core input dicts of the most recent run

# --- engine schedules (tuned against the CoreSim cost model) ---------------
# conv-psum eviction engine per 512-col block, pattern over 32 blocks:
#   A=ACT activation, P=Pool tensor_scalar, D=DVE tensor_scalar
_EV_PAT = "AADAADAADAADAADAADAADAADAADAADAD"
# xy reduction placement per half-plane (32 hp): E=PE DoubleRow, P=Pool tree,
# D=DVE tree
_LAST_E = 30
_XY_PAT = "".join("E" if i in (2,9,16,23,30) else ("D" if i in (6,20) else "P") for i in range(32))
# small xz/xy psum eviction engine rotation
_SM_PAT = "DA"


def _build_nc():
    nc = bacc.Bacc("TRN2", target_bir_lowering=False)
    x27 = nc.dram_tensor("x27", [128, 16384], F16, kind="ExternalInput")
    wkm = nc.dram_tensor("wkm", [128, 128], F16, kind="ExternalInput")
    bias = nc.dram_tensor("bias", [128, 1], F32, kind="ExternalInput")
    drw = nc.dram_tensor("drw", [128, 256], F8, kind="ExternalInput")
    yz_out = nc.dram_tensor("yz_out", [128, 4096], F16, kind="ExternalOutput")
    xz_out = nc.dram_tensor("xz_out", [128, 1024], F16, kind="ExternalOutput")
    xy_out = nc.dram_tensor("xy_out", [128, 1024], F16, kind="ExternalOutput")

    inv = 1.0 / 64.0
    ADD, MAX, MULT = (
        mybir.AluOpType.add,
        mybir.AluOpType.max,
        mybir.AluOpType.mult,
    )

    with tile.TileContext(nc) as tc:
        with tc.tile_pool(name="const", bufs=1) as const_pool, \
             tc.tile_pool(name="xin", bufs=1) as xin_pool, \
             tc.tile_pool(name="cp", bufs=16) as c_pool, \
             tc.tile_pool(name="scr", bufs=4) as scr_pool, \
             tc.tile_pool(name="outs", bufs=1) as out_pool, \
             tc.tile_pool(name="cv", bufs=2, space="PSUM") as cv_pool, \
             tc.tile_pool(name="red", bufs=1, space="PSUM") as red_pool, \
             tc.tile_pool(name="yzp", bufs=1, space="PSUM") as yz_pool:

            xt = xin_pool.tile([128, 16384], F16)
            # first plane-quad h0 half in two small DMAs so conv starts early
            nc.sync.dma_start(out=xt[:, 0:1024], in_=x27[:, 0:1024])
            wt = const_pool.tile([128, 128], F16)
            nc.sync.dma_start(out=wt[:], in_=wkm[:])
            wt_ap = wt[:]
            wpp = wt_ap.ap[0][0]
            bi = const_pool.tile([128, 1], F32)
            nc.sync.dma_start(out=bi[:], in_=bias[:])
            dw = const_pool.tile([128, 256], F8)
            nc.sync.dma_start(out=dw[:], in_=drw[:])
            nc.sync.dma_start(out=xt[:, 1024:2048], in_=x27[:, 1024:2048])
            for h in range(2):
                for u in range(4):
                    if h == 0 and u == 0:
                        continue
                    c0 = u * 4096 + h * 2048
                    nc.sync.dma_start(
                        out=xt[:, c0:c0 + 2048], in_=x27[:, c0:c0 + 2048])
            # preload the ACT Relu table off the critical path
            warm = const_pool.tile([128, 1], F16)
            nc.scalar.activation(
                warm[:], bi[:],
                mybir.ActivationFunctionType.Relu, bias=0.0, scale=1.0)

            xt_ap = xt[:]
            xpp = xt_ap.ap[0][0]
            dw_ap = dw[:]
            dpp = dw_ap.ap[0][0]
            dr_lhs = bass.AP(tensor=dw_ap.tensor, offset=dw_ap.offset,
                             ap=[[dpp, 128], [128, 2], [1, 128]])

            yz_sb = out_pool.tile([128, 4096], F16)  # (h, g1h:32, g2:64)
            xz_sb = out_pool.tile([128, 2048], F16)  # (h, p:16, g2:64)
            xzf = out_pool.tile([128, 1024], F16)    # (p:16, g2:64)
            xy_sb = out_pool.tile([128, 1024], F16)  # (p:16, h, g1h:32)

            def evict(eng, dst, src):
                if eng == "A":
                    nc.scalar.activation(
                        dst, src, mybir.ActivationFunctionType.Relu,
                        bias=bi[:], scale=1.0)
                else:
                    e = nc.gpsimd if eng == "P" else nc.vector
                    e.tensor_scalar(out=dst, in0=src, scalar1=bi[:],
                                    scalar2=0.0, op0=ADD, op1=MAX)

            def evict_scaled(eng, dst, src):
                # psum fp32 -> fp16 with the 1/64 mean fold
                if eng == "A":
                    nc.scalar.activation(
                        dst, src, mybir.ActivationFunctionType.Copy,
                        bias=0.0, scale=inv)
                else:
                    e = nc.gpsimd if eng == "P" else nc.vector
                    e.tensor_scalar(out=dst, in0=src, scalar1=inv,
                                    scalar2=None, op0=MULT)

            def xy_tree(eng, cp_ap, ccols, out_cols):
                # fp8 [128, 32, 64] -> sum over g2 via fp16 pairwise tree
                e = nc.gpsimd if eng == "P" else nc.vector
                s = scr_pool.tile([128, 1536], F16, tag=f"scr{eng}")
                c3 = bass.AP(tensor=cp_ap.tensor, offset=cp_ap.offset + ccols,
                             ap=[[cp_ap.ap[0][0], 128], [64, 32], [1, 64]])
                t0 = s[:, 0:1024].rearrange("q (a b) -> q a b", a=32)
                e.tensor_tensor(out=t0, in0=c3[:, :, 0:32],
                                in1=c3[:, :, 32:64], op=ADD)
                t1 = s[:, 1024:1536].rearrange("q (a b) -> q a b", a=32)
                e.tensor_tensor(out=t1, in0=t0[:, :, 0:16],
                                in1=t0[:, :, 16:32], op=ADD)
                t2 = s[:, 0:256].rearrange("q (a b) -> q a b", a=32)
                e.tensor_tensor(out=t2, in0=t1[:, :, 0:8],
                                in1=t1[:, :, 8:16], op=ADD)
                t3 = s[:, 256:384].rearrange("q (a b) -> q a b", a=32)
                e.tensor_tensor(out=t3, in0=t2[:, :, 0:4],
                                in1=t2[:, :, 4:8], op=ADD)
                t4 = s[:, 384:448].rearrange("q (a b) -> q a b", a=32)
                e.tensor_tensor(out=t4, in0=t3[:, :, 0:2],
                                in1=t3[:, :, 2:4], op=ADD)
                t5 = s[:, 448:480]
                nt4 = s[:, 384:448].rearrange("q (a b) -> q a b", a=32)
                e.tensor_tensor(out=t5, in0=nt4[:, :, 0], in1=nt4[:, :, 1],
                                op=ADD)
                e.tensor_scalar(out=out_cols, in0=t5, scalar1=inv,
                                scalar2=None, op0=MULT)

            # Bresenham A/D rotation for the 64 conv-psum eviction chunks
            EV_ACT = 36
            ev_engines = []
            acc = 0
            for i in range(64):
                acc += EV_ACT
                if acc >= 64:
                    acc -= 64
                    ev_engines.append("A")
                else:
                    ev_engines.append("D")

            ev_i = 0
            sm_engines = None
            pend = []  # deferred per-hp reduction emitters (2-hp lookahead)
            pair_tiles = {}
            for h in range(2):
                for p in range(16):
                    u, a = p // 4, p % 4
                    hp_i = h * 16 + p
                    g8 = hp_i // 8   # 8-hp xz bank group
                    r8 = hp_i % 8
                    if hp_i == 0:
                        xy_bank = red_pool.tile([128, 512], F32, tag="redxy")
                        xy_started = [False]
                    if r8 == 0:
                        xz_bank = red_pool.tile([128, 512], F32, tag="redxz")
                    if p % 2 == 0:
                        cpt = c_pool.tile([128, 4096], F8, tag="cpair")
                        pair_tiles[(h, p // 2)] = cpt
                    cpt_ap = pair_tiles[(h, p // 2)][:]
                    cpp = cpt_ap.ap[0][0]
                    ccols = (p % 2) * 2048

                    for blk2 in range(2):
                        ps = cv_pool.tile([128, 1024], F32, tag="cv")
                        for half in range(2):
                            blk = blk2 * 2 + half
                            rhs = bass.AP(
                                tensor=xt_ap.tensor,
                                offset=xt_ap.offset + 32 * a * xpp + u * 4096
                                + (h * 32 + blk * 8) * 64,
                                ap=[[xpp, 27], [64, 8], [1, 64]],
                            )
                            lhs = bass.AP(
                                tensor=wt_ap.tensor,
                                offset=wt_ap.offset + 32 * a * wpp,
                                ap=[[wpp, 27], [1, 128]],
                            )
                            nc.tensor.matmul(
                                ps[:, half * 512:(half + 1) * 512],
                                lhsT=lhs, rhs=rhs, start=True, stop=True,
                                tile_position=(32 * a, 0))
                        dst = cpt_ap.tensor  # noqa: placeholder
                        dstap = pair_tiles[(h, p // 2)][
                            :, ccols + blk2 * 1024:ccols + (blk2 + 1) * 1024]
                        evict(ev_engines[ev_i % 64], dstap, ps[:])
                        ev_i += 1

                    def make_red(h=h, p=p, cpt_ap=cpt_ap, cpp=cpp,
                                 ccols=ccols, xz_bank=xz_bank,
                                 xy_bank=xy_bank, r8=r8, hp_i=hp_i,
                                 xy_started=xy_started):
                        def emit():
                            nonlocal sm_engines
                            # xz: 16 g1-pair matmuls into the shared 8-hp bank
                            for j in range(16):
                                rhs = bass.AP(
                                    tensor=cpt_ap.tensor,
                                    offset=cpt_ap.offset + ccols + j * 128,
                                    ap=[[cpp, 128], [64, 2], [1, 64]])
                                nc.tensor.matmul(
                                    xz_bank[:, r8 * 64:r8 * 64 + 64],
                                    lhsT=dr_lhs, rhs=rhs,
                                    start=(r8 == 0 and j == 0),
                                    stop=(r8 == 7 and j == 15),
                                    perf_mode=mybir.MatmulPerfMode.DoubleRow)
                            # xy
                            xy_dst = xy_sb[:, p * 64 + h * 32:
                                           p * 64 + h * 32 + 32]
                            mode = _XY_PAT[hp_i]
                            if mode == "E":
                                ecols = _XY_PAT[:hp_i].count("E") * 32
                                for g in range(32):
                                    rhs = bass.AP(
                                        tensor=cpt_ap.tensor,
                                        offset=cpt_ap.offset + ccols + 2 * g,
                                        ap=[[cpp, 128], [1, 2], [64, 32]])
                                    nc.tensor.matmul(
                                        xy_bank[:, ecols:ecols + 32],
                                        lhsT=dr_lhs, rhs=rhs,
                                        start=(g == 0), stop=(g == 31),
                                        perf_mode=mybir.MatmulPerfMode.DoubleRow)
                                evict_scaled("A", xy_dst,
                                             xy_bank[:, ecols:ecols + 32])
                            else:
                                xy_tree(mode, cpt_ap, ccols, xy_dst)
                            # yz part A (cols 0-1023) incremental on odd planes
                            if p % 2 == 1:
                                for ss in range(2):
                                    rhs = bass.AP(
                                        tensor=cpt_ap.tensor,
                                        offset=cpt_ap.offset + ss * 512,
                                        ap=[[cpp, 128], [2048, 2], [1, 512]])
                                    nc.tensor.matmul(
                                        yz_psA[:, ss * 512:(ss + 1) * 512],
                                        lhsT=dr_lhs, rhs=rhs,
                                        start=(p == 1), stop=(p == 15),
                                        perf_mode=mybir.MatmulPerfMode.DoubleRow)
                        return emit

                    if p == 0:
                        yz_psA = yz_pool.tile([128, 1024], F32, tag="yzA")
                    pend.append(make_red())
                    while len(pend) > 2:
                        pend.pop(0)()

                    if r8 == 7:
                        # defer bank eviction behind the lookahead
                        def make_bank_ev(h=h, g8=g8, xz_bank=xz_bank):
                            def emit():
                                p0 = (g8 % 2) * 8
                                evict_scaled(
                                    "A",
                                    xz_sb[:, h * 1024 + p0 * 64:
                                          h * 1024 + p0 * 64 + 512],
                                    xz_bank[:, 0:512])
                            return emit
                        pend.append(make_bank_ev())

                while pend:
                    pend.pop(0)()

                # yz part B (cols 1024-2047): re-read the 8 resident pairs
                ybase = h * 2048
                nc.scalar.activation(
                    yz_sb[:, ybase:ybase + 1024], yz_psA[:],
                    mybir.ActivationFunctionType.Copy, bias=0.0, scale=inv)
                yz_psB = yz_pool.tile([128, 1024], F32, tag="yzA")
                hp_tiles = dict(pair_tiles)
                for k in range(8):
                    cpt_ap = hp_tiles[(h, k)][:]
                    cpp = cpt_ap.ap[0][0]
                    for ss in range(2):
                        rhs = bass.AP(
                            tensor=cpt_ap.tensor,
                            offset=cpt_ap.offset + 1024 + ss * 512,
                            ap=[[cpp, 128], [2048, 2], [1, 512]])
                        nc.tensor.matmul(
                            yz_psB[:, ss * 512:(ss + 1) * 512],
                            lhsT=dr_lhs, rhs=rhs,
                            start=(k == 0), stop=(k == 7),
                            perf_mode=mybir.MatmulPerfMode.DoubleRow)
                nc.vector.tensor_scalar(
                    out=yz_sb[:, ybase + 1024:ybase + 2048],
                    in0=yz_psB[:], scalar1=inv, scalar2=None, op0=MULT)
                nc.sync.dma_start(out=yz_out[:, ybase:ybase + 2048],
                                  in_=yz_sb[:, ybase:ybase + 2048])
                pair_tiles.clear()

            # xz: combine the two g1-halves (already 1/64-scaled)
            nc.vector.tensor_tensor(out=xzf[:], in0=xz_sb[:, 0:1024],
                                    in1=xz_sb[:, 1024:2048], op=ADD)
            nc.sync.dma_start(out=xz_out[:], in_=xzf[:])
            nc.sync.dma_start(out=xy_out[:], in_=xy_sb[:])
    nc.compile()
    return nc


def _host_inputs(x, conv_w, conv_b):
    w27 = np.ascontiguousarray(
        conv_w.reshape(C, 27).T).astype(np.float16)      # [27,128] t=dx*9+dy*3+dz
    wkm = np.zeros((128, 128), np.float16)
    for a in range(4):
        wkm[32 * a:32 * a + 27] = w27
    bias = conv_b.reshape(C, 1).astype(np.float32)
    drw = np.zeros((128, 256), NP_F8)
    idx = np.arange(128)
    drw[idx, idx] = 1.0
    drw[idx, 128 + idx] = 1.0

    in_maps = []
    for core in range(8):
        b, q = core // 4, core % 4
        xe = np.zeros((18, 66, 66), np.float32)
        lo = 16 * q - 1
        s0, e0 = max(lo, 0), min(lo + 18, 64)
        xe[s0 - lo:s0 - lo + (e0 - s0), 1:65, 1:65] = x[b, 0, s0:e0]
        x27 = np.zeros((128, 16384), np.float16)
        for a in range(4):
            for t in range(27):
                dx, r = divmod(t, 9)
                dy, dz = divmod(r, 3)
                row = 32 * a + t
                for u in range(4):
                    p = 4 * u + a
                    x27[row, 4096 * u:4096 * (u + 1)] = (
                        xe[p + dx, dy:dy + 64, dz:dz + 64].reshape(-1))
        in_maps.append({"x27": x27, "wkm": wkm, "bias": bias, "drw": drw})
    return in_maps


def kernel(x, conv_w, conv_b):
    global LAST_RESULTS, LAST_IN_MAPS
    if "nc" not in _CACHE:
        _CACHE["nc"] = _build_nc()
    nc = _CACHE["nc"]

    in_maps = _host_inputs(x, conv_w, conv_b)
    LAST_IN_MAPS = in_maps
    res = run_bass_kernel_spmd(
        nc, in_maps, core_ids=list(range(8)),
        trace=bool(int(os.environ.get("KERNEL_TRACE", "0"))),
    )
    LAST_RESULTS = res

    xz_grid = np.zeros((B, C, 64, 64), np.float32)  # [b, ch, g2, g0]
    xy_grid = np.zeros((B, C, 64, 64), np.float32)  # [b, ch, g1, g0]
    yz_grid = np.zeros((B, C, 64, 64), np.float32)  # [b, ch, g1, g2]
    for core in range(8):
        b, q = core // 4, core % 4
        r = res.results[core]
        xz = r["xz_out"].astype(np.float32).reshape(C, 16, 64)  # [ch,p,g2]
        xz_grid[b, :, :, 16 * q:16 * q + 16] = xz.transpose(0, 2, 1)
        xy = r["xy_out"].astype(np.float32).reshape(C, 16, 64)  # [ch,p,g1]
        xy_grid[b, :, :, 16 * q:16 * q + 16] = xy.transpose(0, 2, 1)
        yz = r["yz_out"].astype(np.float32).reshape(C, 64, 64)  # [ch,g1,g2]
        yz_grid[b] += yz

    fea_xz = np.zeros((B, C, RESO, RESO), np.float32)
    fea_xy = np.zeros((B, C, RESO, RESO), np.float32)
    fea_yz = np.zeros((B, C, RESO, RESO), np.float32)
    rows, cols = U[:, None], U[None, :]
    fea_xz[:, :, rows, cols] = xz_grid
    fea_xy[:, :, rows, cols] = xy_grid
    fea_yz[:, :, rows, cols] = yz_grid.transpose(0, 1, 3, 2)
    return (fea_xz, fea_xy, fea_yz)
